# revision 1
# baseline (speedup 1.0000x reference)
"""Multi-head attention + residual + LayerNorm on 8 TRN2 NeuronCores.

Problem (fixed shapes): B=2, S=2048, D=1024, H=16 heads, head_dim=64.
    q,k,v = x@Wq+bq, x@Wk+bk, x@Wv+bv   (per-head split)
    probs = softmax(q@k^T/8 + mask); ctx = probs@v
    out = LayerNorm(ctx@Wo + bo + x) * gamma + beta

Sharding:
  Stage A (tensor-parallel over heads): core c owns heads {2c, 2c+1}.
    Computes its heads' q/k/v projections, attention, and the normalized
    per-head context, laid out transposed: ctxT [head_feat, B*S] (bf16).
  Stage B (data-parallel over rows): core c owns rows [512c, 512(c+1)) of
    the flattened [4096, 1024] activations. Computes ctx@Wo + bo + x and
    the LayerNorm over the feature dim.
  Host only reshuffles arrays between stages (no FLOPs beyond x+bo).

All matmuls run in bf16 (fp32 PSUM accumulation); softmax exp runs on the
scalar engine in fp32 from PSUM with the 1/sqrt(64) scale and additive mask
folded into the activation's scale/bias. The softmax denominator comes for
free from a 65th all-ones column appended to v (so ctx matmuls produce
[sum_j p_j * v_j ; sum_j p_j] in one accumulation), avoiding any
cross-partition reduction.
"""

import numpy as np
import ml_dtypes

import concourse.bacc as bacc
import concourse.bass as bass
import concourse.tile as tile
from concourse import mybir
from concourse.bass_utils import run_bass_kernel_spmd

BF16 = ml_dtypes.bfloat16

B, S, D, H = 2, 2048, 1024, 16
HD = D // H          # 64
NCORES = 8
HPC = H // NCORES    # 2 heads per core
R = B * S            # 4096 rows
RPC = R // NCORES    # 512 rows per core in stage B
KT = D // 128        # 8 contraction tiles
LN_EPS = 1e-12

_cache = {}

# Set by callers that want HW exec times (requires the NTFF shim).
PROFILE = False
last_exec_ns = {}


def _build_stage_a():
    nc = bacc.Bacc("TRN2", target_bir_lowering=False, debug=False,
                   num_devices=NCORES)
    f32 = mybir.dt.float32
    bf16 = mybir.dt.bfloat16

    # All inputs are pre-arranged on the host to match SBUF layouts, so
    # every load is large contiguous segments (fast DMA, few descriptors).
    xts = nc.dram_tensor("xts", [R // 512, 128, KT, 512], bf16,
                         kind="ExternalInput").ap()
    wqk = nc.dram_tensor("wqk", [128, KT, 256], bf16,
                         kind="ExternalInput").ap()
    wv = nc.dram_tensor("wv", [128, KT, 128], bf16,
                        kind="ExternalInput").ap()
    bqk = nc.dram_tensor("bqk", [128, 2], f32, kind="ExternalInput").ap()
    bvp = nc.dram_tensor("bv", [128], f32, kind="ExternalInput").ap()
    mask = nc.dram_tensor("mask", [128, B, S // 128], f32,
                          kind="ExternalInput").ap()
    ctxT = nc.dram_tensor("ctxT", [HPC, HD, R], bf16, kind="ExternalOutput").ap()

    with tile.TileContext(nc) as tc:
        with (
            tc.tile_pool(name="singles", bufs=1) as singles,
            tc.tile_pool(name="xt", bufs=5) as xt_pool,
            tc.tile_pool(name="mixps", bufs=4, space="PSUM") as mix_psum,
            tc.tile_pool(name="scps", bufs=2, space="PSUM") as sc_psum,
            tc.tile_pool(name="expp", bufs=3) as exp_pool,
            tc.tile_pool(name="norm", bufs=6) as norm_pool,
        ):
            # --- resident tensors ---
            w_qk_sb = singles.tile([128, KT, 256], bf16)
            nc.scalar.dma_start(out=w_qk_sb, in_=wqk[:])
            w_v_sb = singles.tile([128, KT, 128], bf16)
            nc.scalar.dma_start(out=w_v_sb, in_=wv[:])
            bqk_sb = singles.tile([128, 2], f32)
            nc.scalar.dma_start(out=bqk_sb, in_=bqk[:])
            bv_sb = singles.tile([128, 128], f32)
            nc.scalar.dma_start(
                out=bv_sb,
                in_=bass.AP(tensor=bvp.tensor, offset=bvp.offset,
                            ap=[[0, 128]] + list(bvp.ap)))
            mask_sb = singles.tile([128, B, S // 128], f32)
            nc.scalar.dma_start(out=mask_sb, in_=mask[:])

            q_sb = singles.tile([128, R], bf16)   # [head_feat, row]
            k_sb = singles.tile([128, R], bf16)
            # v in natural layout + 65th all-ones column per (rowtile, head)
            v_all = singles.tile([128, R // 128, HPC, HD + 1], bf16)
            nc.vector.memset(v_all[:, :, :, HD:HD + 1], 1.0)

            # Matmuls below split their K=128 contraction into two K=64
            # halves on disjoint PE row groups (partitions 0-63 / 64-127).
            # The halves' MATMULs run concurrently on the array, and each
            # half's LDWEIGHTS overlaps the other half's in-flight MATMUL
            # (row groups don't conflict), hiding weight-load time.
            JT = S // 128  # 16 key tiles per batch

            def qkv_pieces(rb):
                """qkv for one row block as 4 small closures (~2.5us of PE
                each) so they can slot into attention's PE slack without
                starving the exp pipeline."""
                xt = xt_pool.tile([128, KT, 512], bf16, tag="xt",
                                  name=f"xt_{rb}")

                def load():
                    e1 = nc.sync if rb % 2 == 0 else nc.gpsimd
                    e2 = (nc.scalar if rb < 4 else
                          (nc.gpsimd if rb % 2 == 0 else nc.sync))
                    e1.dma_start(out=xt[:, 0:KT // 2, :],
                                 in_=xts[rb, :, 0:KT // 2, :])
                    e2.dma_start(out=xt[:, KT // 2:KT, :],
                                 in_=xts[rb, :, KT // 2:KT, :])

                ps_hold = {}

                def proj_half(w_lo, w_sb, bias, half):
                    def _p():
                        if half == 0:
                            ps_hold[w_lo] = mix_psum.tile(
                                [128, 512], mybir.dt.float32, tag="mix",
                                name=f"pqk_{rb}_{w_lo}")
                        ps = ps_hold[w_lo]
                        for kt in range(half * 4, half * 4 + 4):
                            nc.tensor.matmul(
                                ps,
                                lhsT=w_qk_sb[:, kt, w_lo:w_lo + 128],
                                rhs=xt[:, kt, :],
                                start=(kt == 0), stop=(kt == KT - 1))
                        if half == 1:
                            nc.vector.tensor_scalar_add(
                                out=w_sb[:, rb * 512:(rb + 1) * 512],
                                in0=ps, scalar1=bias)
                    return _p

                def vproj(rt):
                    def _v():
                        psv = mix_psum.tile([128, 128], mybir.dt.float32,
                                            tag="mix", name=f"psv_{rb}_{rt}")
                        for kt in range(KT):
                            nc.tensor.matmul(
                                psv,
                                lhsT=xt[:, kt, rt * 128:(rt + 1) * 128],
                                rhs=w_v_sb[:, kt, :],
                                start=(kt == 0), stop=(kt == KT - 1))
                        for h in range(HPC):
                            nc.vector.tensor_add(
                                out=v_all[:, rb * 4 + rt, h, 0:HD],
                                in0=psv[:, h * HD:(h + 1) * HD],
                                in1=bv_sb[:, h * HD:(h + 1) * HD])
                    return _v

                load()
                return [proj_half(0, q_sb, bqk_sb[:, 0:1], 0),
                        proj_half(0, q_sb, bqk_sb[:, 0:1], 1),
                        proj_half(128, k_sb, bqk_sb[:, 1:2], 0),
                        proj_half(128, k_sb, bqk_sb[:, 1:2], 1),
                        vproj(0), vproj(1), vproj(2), vproj(3)]

            def emit_qkv(rb):
                for piece in qkv_pieces(rb):
                    piece()

            def emit_attention(b, ib, fillers=()):
                q_lo = b * S + ib * 512
                ctx_ps = [mix_psum.tile([HD + 1, 512], mybir.dt.float32,
                                        tag="mix", name=f"ctx_{b}_{ib}_{h}")
                          for h in range(HPC)]
                fillers = list(fillers)
                for jt in range(JT):
                    if fillers and jt % 2 == 1:
                        fillers.pop(0)()
                    k_lo = b * S + jt * 128
                    sc = sc_psum.tile([128, HPC, 512], mybir.dt.float32,
                                      tag="sc", name=f"sc_{b}_{ib}_{jt}")
                    for h in range(HPC):
                        nc.tensor.matmul(
                            sc[:, h, :],
                            lhsT=k_sb[h * HD:(h + 1) * HD, k_lo:k_lo + 128],
                            rhs=q_sb[h * HD:(h + 1) * HD, q_lo:q_lo + 512],
                            start=True, stop=True)
                    ex = exp_pool.tile([128, HPC, 512], bf16, tag="ex",
                                       name=f"ex_{b}_{ib}_{jt}")
                    nc.scalar.activation(
                        out=ex, in_=sc,
                        func=mybir.ActivationFunctionType.Exp,
                        bias=mask_sb[:, b, jt:jt + 1], scale=0.125)
                    for h in range(HPC):
                        nc.tensor.matmul(
                            ctx_ps[h],
                            lhsT=v_all[:, b * JT + jt, h, :],
                            rhs=ex[:, h, :],
                            start=(jt == 0), stop=(jt == JT - 1))
                # Copy ctx PSUM to SBUF right away: releasing the PSUM slot
                # quickly keeps the next block's matmuls from stalling the
                # PE FIFO behind the (longer) normalize chain below.
                css = []
                for h in range(HPC):
                    cs = norm_pool.tile([HD + 1, 512], mybir.dt.float32,
                                        tag="cs")
                    nc.vector.tensor_copy(out=cs, in_=ctx_ps[h])
                    css.append(cs)
                # normalize by the ones-column denominator (partition HD)
                for h in range(HPC):
                    cs = css[h]
                    den = norm_pool.tile([1, 512], mybir.dt.float32,
                                         tag="den")
                    nc.vector.tensor_copy(out=den, in_=cs[HD:HD + 1, :])
                    rec = norm_pool.tile([1, 512], mybir.dt.float32,
                                         tag="rec")
                    # ~18-bit reciprocal is plenty for softmax denoms
                    # (strictly positive, well-scaled); 5x faster on DVE.
                    nc.vector.reciprocal_approx_fast(out=rec, in_=den)
                    bc = norm_pool.tile([HD, 512], mybir.dt.float32,
                                        tag="bc")
                    nc.gpsimd.partition_broadcast(bc, rec)
                    ob = norm_pool.tile([HD, 512], bf16, tag="ob")
                    nc.vector.tensor_mul(out=ob, in0=cs[0:HD, :], in1=bc)
                    nc.sync.dma_start(
                        out=ctxT[h, :, q_lo:q_lo + 512], in_=ob)

            # Batch 0's projections, then batch 0's attention with batch 1's
            # projections sliced into the attention blocks' PE slack, then
            # batch 1's attention.
            for rb in range(4):
                emit_qkv(rb)
            for ib in range(4):
                emit_attention(0, ib, fillers=qkv_pieces(4 + ib))
            for ib in range(4):
                emit_attention(1, ib)

    nc.compile()
    return nc


def _build_stage_b(gamma_trivial=False, beta_trivial=False):
    nc = bacc.Bacc("TRN2", target_bir_lowering=False, debug=False,
                   num_devices=NCORES)
    f32 = mybir.dt.float32
    bf16 = mybir.dt.bfloat16

    ctr = nc.dram_tensor("ctr", [128, KT, RPC], bf16,
                         kind="ExternalInput").ap()
    wo = nc.dram_tensor("wo", [128, KT, D], bf16, kind="ExternalInput").ap()
    xpb = nc.dram_tensor("xpb", [RPC, D], f32, kind="ExternalInput").ap()
    gamma = nc.dram_tensor("gamma", [D], f32, kind="ExternalInput").ap()
    beta = nc.dram_tensor("beta", [D], f32, kind="ExternalInput").ap()
    out = nc.dram_tensor("out", [RPC, D], f32, kind="ExternalOutput").ap()

    with tile.TileContext(nc) as tc:
        with (
            tc.tile_pool(name="singles", bufs=1) as singles,
            tc.tile_pool(name="xp", bufs=2) as xp_pool,
            tc.tile_pool(name="hid", bufs=2) as h_pool,
            tc.tile_pool(name="ps", bufs=2, space="PSUM") as ps_pool,
            tc.tile_pool(name="stat", bufs=8) as stat_pool,
            tc.tile_pool(name="outp", bufs=2) as out_pool,
        ):
            engs = [nc.sync, nc.scalar, nc.gpsimd]
            wo_sb = singles.tile([128, KT, D], bf16)
            ct_sb = singles.tile([128, KT, RPC], bf16)
            for kt in range(KT):
                engs[kt % 3].dma_start(out=ct_sb[:, kt, :], in_=ctr[:, kt, :])
                engs[(kt + 1) % 3].dma_start(out=wo_sb[:, kt, :],
                                             in_=wo[:, kt, :])
            gm_sb = singles.tile([128, D], f32)
            nc.sync.dma_start(
                out=gm_sb,
                in_=bass.AP(tensor=gamma.tensor, offset=gamma.offset,
                            ap=[[0, 128]] + list(gamma.ap)))
            bt_sb = singles.tile([128, D], f32)
            nc.sync.dma_start(
                out=bt_sb,
                in_=bass.AP(tensor=beta.tensor, offset=beta.offset,
                            ap=[[0, 128]] + list(beta.ap)))
            eps_sb = singles.tile([128, 1], f32)
            nc.vector.memset(eps_sb, LN_EPS)

            MT = RPC // 128  # 4 row tiles
            for mt in range(MT):
                xp = xp_pool.tile([128, D], f32)
                nc.sync.dma_start(out=xp,
                                  in_=xpb[mt * 128:(mt + 1) * 128, :])
                hid = h_pool.tile([128, D], f32)
                for nb in range(2):
                    ps = ps_pool.tile([128, 512], mybir.dt.float32)
                    for kt in range(KT):
                        nc.tensor.matmul(
                            ps,
                            lhsT=ct_sb[:, kt, mt * 128:(mt + 1) * 128],
                            rhs=wo_sb[:, kt, nb * 512:(nb + 1) * 512],
                            start=(kt == 0), stop=(kt == KT - 1))
                    nc.vector.tensor_add(out=hid[:, nb * 512:(nb + 1) * 512],
                                         in0=ps,
                                         in1=xp[:, nb * 512:(nb + 1) * 512])
                # LayerNorm over the 1024-wide free dim
                st = stat_pool.tile([128, 2, 6], f32, tag="st")
                for g in range(2):
                    nc.vector.bn_stats(out=st[:, g, :],
                                       in_=hid[:, g * 512:(g + 1) * 512])
                mv = stat_pool.tile([128, 2], f32, tag="mv")
                nc.vector.bn_aggr(out=mv, in_=st)
                sd = stat_pool.tile([128, 1], f32, tag="sd")
                nc.scalar.activation(out=sd, in_=mv[:, 1:2],
                                     func=mybir.ActivationFunctionType.Sqrt,
                                     bias=eps_sb, scale=1.0)
                rs = stat_pool.tile([128, 1], f32, tag="rs")
                nc.vector.reciprocal(out=rs, in_=sd)
                ot = out_pool.tile([128, D], f32)
                nc.vector.tensor_scalar(out=ot, in0=hid,
                                        scalar1=mv[:, 0:1], scalar2=rs,
                                        op0=mybir.AluOpType.subtract,
                                        op1=mybir.AluOpType.mult)
                if not gamma_trivial:
                    nc.vector.tensor_mul(out=ot, in0=ot, in1=gm_sb)
                if not beta_trivial:
                    nc.vector.tensor_add(out=ot, in0=ot, in1=bt_sb)
                engs[mt % 3].dma_start(
                    out=out[mt * 128:(mt + 1) * 128, :], in_=ot)

    nc.compile()
    return nc


def _get(name, **kw):
    key = (name, tuple(sorted(kw.items())))
    if key not in _cache:
        _cache[key] = (_build_stage_a() if name == "a"
                       else _build_stage_b(**kw))
    return _cache[key]


def _run(nc, in_maps, label):
    kwargs = {}
    if PROFILE:
        kwargs = dict(trace=True)
    res = run_bass_kernel_spmd(nc, in_maps, list(range(NCORES)), **kwargs)
    if PROFILE:
        last_exec_ns[label] = res.exec_time_ns
    return res.results


def prep_a(inputs):
    x = np.asarray(inputs["input_tensor"], dtype=np.float32)
    mask = np.asarray(inputs["attention_mask"], dtype=np.float32)[:, 0, 0, :]
    Wq = np.asarray(inputs["Wq"], dtype=np.float32)
    bq = np.asarray(inputs["bq"], dtype=np.float32)
    Wk = np.asarray(inputs["Wk"], dtype=np.float32)
    bk = np.asarray(inputs["bk"], dtype=np.float32)
    Wv = np.asarray(inputs["Wv"], dtype=np.float32)
    bv = np.asarray(inputs["bv"], dtype=np.float32)
    xf = x.reshape(R, D)
    xts = np.ascontiguousarray(
        xf.reshape(R // 512, 512, KT, 128).transpose(0, 3, 2, 1)).astype(BF16)
    mask_h = np.ascontiguousarray(
        mask.reshape(B, S // 128, 128).transpose(2, 0, 1))
    in_maps_a = []
    for c in range(NCORES):
        cs = slice(c * 128, (c + 1) * 128)
        wqk_c = np.concatenate([Wq[:, cs], Wk[:, cs]], axis=1)
        in_maps_a.append({
            "xts": xts,
            "wqk": np.ascontiguousarray(
                wqk_c.reshape(KT, 128, 256).transpose(1, 0, 2)).astype(BF16),
            "wv": np.ascontiguousarray(
                Wv[:, cs].reshape(KT, 128, 128).transpose(1, 0, 2)
            ).astype(BF16),
            "bqk": np.ascontiguousarray(
                np.stack([bq[cs], bk[cs]], axis=1)).astype(np.float32),
            "bv": np.ascontiguousarray(bv[cs]),
            "mask": mask_h,
        })
    return in_maps_a, xf


def kernel(**inputs):
    x = np.asarray(inputs["input_tensor"], dtype=np.float32)
    mask = np.asarray(inputs["attention_mask"], dtype=np.float32)[:, 0, 0, :]
    Wq = np.asarray(inputs["Wq"], dtype=np.float32)
    bq = np.asarray(inputs["bq"], dtype=np.float32)
    Wk = np.asarray(inputs["Wk"], dtype=np.float32)
    bk = np.asarray(inputs["bk"], dtype=np.float32)
    Wv = np.asarray(inputs["Wv"], dtype=np.float32)
    bv = np.asarray(inputs["bv"], dtype=np.float32)
    Wo = np.asarray(inputs["Wo"], dtype=np.float32)
    bo = np.asarray(inputs["bo"], dtype=np.float32)
    gamma = np.asarray(inputs["ln_gamma"], dtype=np.float32)
    beta = np.asarray(inputs["ln_beta"], dtype=np.float32)

    in_maps_a, xf = prep_a(inputs)
    res_a = _run(_get("a"), in_maps_a, "stage_a")

    ctxT_full = np.empty((D, R), dtype=BF16)
    for c in range(NCORES):
        ctxT_full[c * 128:(c + 1) * 128] = res_a[c]["ctxT"].reshape(128, R)

    wo_b = np.ascontiguousarray(
        Wo.reshape(KT, 128, D).transpose(1, 0, 2)).astype(BF16)
    xpb_f = xf + bo[None, :]
    in_maps_b = []
    for c in range(NCORES):
        rs = slice(c * RPC, (c + 1) * RPC)
        in_maps_b.append({
            "ctr": np.ascontiguousarray(
                ctxT_full[:, rs].reshape(KT, 128, RPC).transpose(1, 0, 2)),
            "wo": wo_b,
            "xpb": np.ascontiguousarray(xpb_f[rs]),
            "gamma": gamma,
            "beta": beta,
        })
    res_b = _run(_get("b", gamma_trivial=bool(np.all(gamma == 1.0)),
                      beta_trivial=bool(np.all(beta == 0.0))),
                 in_maps_b, "stage_b")

    out = np.concatenate([res_b[c]["out"] for c in range(NCORES)], axis=0)
    return out.reshape(B, S, D)

